# revision 14
# baseline (speedup 1.0000x reference)
"""Trainium2 Bass kernel for nn_MultiHeadSelector (topk_masking).

Contract: kernel(**inputs) takes FULL inputs (numpy), returns the FULL
output tuple (hidden_states, selected_hidden, patch_idx) exactly like the
reference. Internally: pure data parallelism, batch b -> NeuronCore b
(B == 8 == n_cores). Only x[:, :, 0, 1:] of the big attention tensor is
mathematically live; it is sliced on the host and shipped per-core.

Key algebraic facts used on-device (validated against the reference):
  * top-84 selection per head == (score >= 84th-largest) [no boundary ties]
  * bincount of topk indices == sum over heads of the selection mask
  * the GCN adjacency pw pw^T is rank-1, and only the anchor row of the
    GCN output is consumed, so both 784x784 matmuls collapse to dot
    products with pw:
        u1 = (sum_s pw*dist, sum_s pw*ang) @ gc1_w
        u2 = (sum pw^2) * relu(u1) @ gc2_w
        delta = leaky_relu(pw[anchor] * u2, 0.2)
  * count-sort key 2048*count - s reproduces argsort(-count) stable order
    (equal counts -> ascending s) and s is exactly recoverable in f32.
"""

import os
from contextlib import ExitStack

import numpy as np

import concourse.bass as bass
import concourse.tile as tile
from concourse import bacc, mybir
from concourse import bass_isa
from concourse.bass_utils import run_bass_kernel_spmd

F32 = mybir.dt.float32
I32 = mybir.dt.int32
ALU = mybir.AluOpType
ACT = mybir.ActivationFunctionType
AX = mybir.AxisListType

B, C, S, H28, P = 8, 12, 784, 28, 84
NCHUNK, CH = 8, 98            # score/key extraction chunking: s-contig / s-interleave
SCORE_ROUNDS = 4              # top-32 per contiguous chunk (observed max need: 22)
KEY_ROUNDS = 3                # top-24 per interleaved chunk (observed max need: 16)
NEG = -1.0e30
RND = 8388608.0               # 2^23, float round-to-nearest trick
PI = float(np.pi)


def build_nc():
    nc = bacc.Bacc("TRN2", target_bir_lowering=False, debug=False,
                   enable_asserts=False, num_devices=8)

    d_score = nc.dram_tensor("score", [C, S], F32, kind="ExternalInput")
    d_score96 = nc.dram_tensor("score96", [C * NCHUNK, CH], F32, kind="ExternalInput")
    d_hidden = nc.dram_tensor("hidden", [S + 1, 768], F32, kind="ExternalInput")
    d_gc1 = nc.dram_tensor("gc1w", [2, 512], F32, kind="ExternalInput")
    d_gc2 = nc.dram_tensor("gc2w", [512, 768], F32, kind="ExternalInput")

    d_oh = nc.dram_tensor("out_hidden", [S + 1, 768], F32, kind="ExternalOutput")
    d_os = nc.dram_tensor("out_sel", [P, 768], F32, kind="ExternalOutput")
    d_op = nc.dram_tensor("out_patch", [1, P], I32, kind="ExternalOutput")

    d_scr = nc.dram_tensor("scr", [1, 4096], F32, kind="Internal")

    with tile.TileContext(nc) as tc, ExitStack() as ctx:
        const = ctx.enter_context(tc.tile_pool(name="const", bufs=1))
        wk = ctx.enter_context(tc.tile_pool(name="wk", bufs=1))
        big = ctx.enter_context(tc.tile_pool(name="big", bufs=1))
        ps = ctx.enter_context(tc.tile_pool(name="ps", bufs=1, space="PSUM"))

        V, SC, PE, GP, SY = nc.vector, nc.scalar, nc.tensor, nc.gpsimd, nc.sync

        # ---------------- constants (no DMA needed) ----------------
        s_i32 = const.tile([H28, H28], I32, tag="s_i32")
        GP.iota(s_i32[:], [[1, H28]], channel_multiplier=H28)       # 28*i + j
        j_i32 = const.tile([H28, H28], I32, tag="j_i32")
        GP.iota(j_i32[:], [[1, H28]], channel_multiplier=0)         # j
        i_i32 = const.tile([H28, H28], I32, tag="i_i32")
        GP.iota(i_i32[:], [[0, H28]], channel_multiplier=1)         # i
        iconst = const.tile([H28, H28], F32, tag="iconst")
        V.tensor_copy(iconst[:], i_i32[:])
        jconst = const.tile([H28, H28], F32, tag="jconst")
        V.tensor_copy(jconst[:], j_i32[:])

        s8_i32 = const.tile([NCHUNK, CH], I32, tag="s8_i32")
        GP.iota(s8_i32[:], [[NCHUNK, CH]], channel_multiplier=1)    # s = 8f + p
        s8const = const.tile([NCHUNK, CH], F32, tag="s8const")
        V.tensor_copy(s8const[:], s8_i32[:])

        p_i32 = const.tile([128, 1], I32, tag="p_i32")
        GP.iota(p_i32[:], [[0, 1]], channel_multiplier=1)
        if128 = const.tile([128, 1], F32, tag="if128")
        V.tensor_copy(if128[:], p_i32[:])

        ones12 = const.tile([C, 1], F32, tag="ones12")
        V.memset(ones12[:], 1.0)
        w12 = const.tile([C, 1], F32, tag="w12")
        V.memset(w12[:], 1.0 / 12.0)

        # banded matrix for the vertical [1,2,1] conv as a PE matmul:
        # M[p, i] = [1,2,1][p-i] (0 outside), p in [0,30), i in [0,28)
        d_i32 = const.tile([30, H28], I32, tag="d_i32")
        GP.iota(d_i32[:], [[-1, H28]], channel_multiplier=1)
        dff = const.tile([30, H28], F32, tag="dff")
        V.tensor_copy(dff[:], d_i32[:])
        b0 = const.tile([30, H28], F32, tag="b0")
        V.tensor_scalar(b0[:], dff[:], 0.0, None, ALU.is_equal)
        b1 = const.tile([30, H28], F32, tag="b1")
        V.tensor_scalar(b1[:], dff[:], 1.0, 2.0, ALU.is_equal, ALU.mult)
        b2 = const.tile([30, H28], F32, tag="b2")
        V.tensor_scalar(b2[:], dff[:], 2.0, None, ALU.is_equal)
        m30 = const.tile([30, H28], F32, tag="m30")
        V.tensor_add(m30[:], b0[:], b1[:])
        V.tensor_add(m30[:], m30[:], b2[:])

        # ---------------- input DMAs ----------------
        sc12 = wk.tile([C, S], F32, tag="sc12")
        SY.dma_start(out=sc12[:], in_=d_score.ap())
        sw96 = wk.tile([C * NCHUNK, CH], F32, tag="sw96")
        SY.dma_start(out=sw96[:], in_=d_score96.ap())

        g1sb = wk.tile([2, 512], F32, tag="g1sb")
        SY.dma_start(out=g1sb[:], in_=d_gc1.ap())

        g2c = []
        for c in range(4):
            t = big.tile([128, 768], F32, tag=f"g2c{c}")
            SY.dma_start(out=t[:], in_=d_gc2.ap()[128 * c:128 * (c + 1), :])
            g2c.append(t)

        hk = [128, 128, 128, 128, 128, 128, 17]
        hid = []
        for c in range(7):
            t = big.tile([hk[c], 768], F32, tag=f"hid{c}")
            SY.dma_start(out=t[:], in_=d_hidden.ap()[128 * c:128 * c + hk[c], :])
            hid.append(t)

        # hidden passthrough (rows 1..784 unchanged; row 0 written later):
        # direct DRAM->DRAM copy, never touches SBUF
        SY.dma_start(out=d_oh.ap()[1:S + 1, :], in_=d_hidden.ap()[1:S + 1, :])

        # ---------------- per-head top-84 threshold ----------------
        # stage 1: top-32 of each 98-elem contiguous chunk (96 rows in parallel)
        cand96 = wk.tile([C * NCHUNK, 8 * SCORE_ROUNDS], F32, tag="cand96")
        for r in range(SCORE_ROUNDS):
            V.max(cand96[:, 8 * r:8 * r + 8], sw96[:])
            if r < SCORE_ROUNDS - 1:
                V.match_replace(sw96[:], cand96[:, 8 * r:8 * r + 8], sw96[:], NEG)
        # compact candidates per head via DRAM bounce: [96, 32] -> [12, 256]
        # (row-major dump is already per-head contiguous: p = 8c + k)
        W = 8 * SCORE_ROUNDS
        scr_c_w = d_scr.ap()[0:1, 1024:1024 + 96 * W].rearrange(
            "a (p f) -> (a p) f", p=C * NCHUNK)
        SY.dma_start(out=scr_c_w, in_=cand96[:])
        cand12 = wk.tile([C, NCHUNK * W], F32, tag="cand12")
        scr_c_r = d_scr.ap()[0:1, 1024:1024 + 96 * W].rearrange(
            "a (c f) -> (a c) f", c=C)
        SY.dma_start(out=cand12[:], in_=scr_c_r)
        # stage 2: 11 rounds of 8 -> threshold = 84th largest per head
        rv12 = wk.tile([C, 8], F32, tag="rv12")
        for r in range(11):
            V.max(rv12[:], cand12[:])
            if r < 10:
                V.match_replace(cand12[:], rv12[:], cand12[:], NEG)
        thr12 = wk.tile([C, 1], F32, tag="thr12")
        V.tensor_copy(thr12[:], rv12[:, 3:4])

        # select mask, new_score
        sel12 = wk.tile([C, S], F32, tag="sel12")
        V.tensor_scalar(sel12[:], sc12[:], thr12[:], None, ALU.is_ge)
        f12 = wk.tile([C, S], F32, tag="f12")
        V.tensor_scalar(f12[:], sel12[:], 0.3, 0.7, ALU.mult, ALU.add)
        ns12 = wk.tile([C, S], F32, tag="ns12")
        V.tensor_mul(ns12[:], f12[:], sc12[:])

        # ---------------- column sums via PE ----------------
        cnt_ps = ps.tile([1, S], F32, tag="ps_a")
        PE.matmul(cnt_ps[:, 0:512], lhsT=ones12[:], rhs=sel12[:, 0:512],
                  start=True, stop=True)
        PE.matmul(cnt_ps[:, 512:S], lhsT=ones12[:], rhs=sel12[:, 512:S],
                  start=True, stop=True)
        pw_ps = ps.tile([1, S], F32, tag="ps_b")
        PE.matmul(pw_ps[:, 0:512], lhsT=w12[:], rhs=ns12[:, 0:512],
                  start=True, stop=True)
        PE.matmul(pw_ps[:, 512:S], lhsT=w12[:], rhs=ns12[:, 512:S],
                  start=True, stop=True)

        # ---------------- 3x3 separable conv on count image ----------------
        cnt_sb = wk.tile([1, S], F32, tag="cnt_sb")
        V.tensor_copy(cnt_sb[:], cnt_ps[:])
        P1 = wk.tile([30, 30], F32, tag="P1")
        V.memset(P1[:], 0.0)
        SY.dma_start(out=P1[1:15, 1:29], in_=cnt_sb[:, 0:392])
        SY.dma_start(out=P1[15:29, 1:29], in_=cnt_sb[:, 392:784])
        t1 = wk.tile([30, 28], F32, tag="t1")
        V.tensor_add(t1[:], P1[:, 0:28], P1[:, 2:30])
        A1t = wk.tile([30, 28], F32, tag="A1t")
        V.scalar_tensor_tensor(A1t[:], P1[:, 1:29], 2.0, t1[:], ALU.mult, ALU.add)
        cc_ps = ps.tile([H28, H28], F32, tag="ps_c")
        PE.matmul(cc_ps[:], lhsT=m30[:], rhs=A1t[:], start=True, stop=True)
        cc28 = wk.tile([H28, H28], F32, tag="cc28")
        V.tensor_copy(cc28[:], cc_ps[:])

        # bounce through DRAM to get the s-interleaved [8, 98] layout
        scr_img = d_scr.ap()[0:1, 0:S].rearrange("a (p f) -> (a p) f", p=H28)
        SY.dma_start(out=scr_img, in_=cc28[:])
        scr_il = d_scr.ap()[0:1, 0:S].rearrange("a (f p) -> (a p) f", p=NCHUNK)
        K8 = wk.tile([NCHUNK, CH], F32, tag="K8")
        SY.dma_start(out=K8[:], in_=scr_il)
        key8 = wk.tile([NCHUNK, CH], F32, tag="key8")
        V.scalar_tensor_tensor(key8[:], K8[:], 2048.0, s8const[:],
                               ALU.mult, ALU.subtract)

        # top-24 per interleaved chunk
        candK = wk.tile([NCHUNK, 8 * KEY_ROUNDS], F32, tag="candK")
        for r in range(KEY_ROUNDS):
            V.max(candK[:, 8 * r:8 * r + 8], key8[:])
            if r < KEY_ROUNDS - 1:
                V.match_replace(key8[:], candK[:, 8 * r:8 * r + 8], key8[:], NEG)
        WK_ = 8 * KEY_ROUNDS
        scr_ck_w = d_scr.ap()[0:1, 800:800 + NCHUNK * WK_].rearrange(
            "a (p f) -> (a p) f", p=NCHUNK)
        SY.dma_start(out=scr_ck_w, in_=candK[:])
        ck1 = wk.tile([1, NCHUNK * WK_], F32, tag="ck1")
        SY.dma_start(out=ck1[:], in_=d_scr.ap()[0:1, 800:800 + NCHUNK * WK_])

        # global top-88 (sorted desc) of the keys
        keys88 = wk.tile([1, 88], F32, tag="keys88")
        for r in range(11):
            V.max(keys88[:, 8 * r:8 * r + 8], ck1[:])
            if r < 10:
                V.match_replace(ck1[:], keys88[:, 8 * r:8 * r + 8], ck1[:], NEG)

        # recover s: key = 2048*cc - s ; cc = round(key/2048) (frac <= .38)
        y84 = wk.tile([1, P], F32, tag="y84")
        V.tensor_scalar(y84[:], keys88[:, 0:P], 1.0 / 2048.0, None, ALU.mult)
        yr = wk.tile([1, P], F32, tag="yr")
        V.tensor_scalar(yr[:], y84[:], RND, None, ALU.add)
        r84 = wk.tile([1, P], F32, tag="r84")
        V.tensor_scalar(r84[:], yr[:], -RND, None, ALU.add)
        sfl = wk.tile([1, P], F32, tag="sfl")
        V.scalar_tensor_tensor(sfl[:], r84[:], 2048.0, keys88[:, 0:P],
                               ALU.mult, ALU.subtract)
        pidxf = wk.tile([1, P], F32, tag="pidxf")
        V.tensor_scalar(pidxf[:], sfl[:], 1.0, None, ALU.add)
        pint = wk.tile([1, P], I32, tag="pint")
        V.tensor_copy(pint[:], pidxf[:])
        SY.dma_start(out=d_op.ap(), in_=pint[:])

        # ---------------- gather selected rows via 0/1 matmul ----------------
        patchB = wk.tile([128, P], F32, tag="patchB")
        GP.partition_broadcast(patchB[:], pidxf[:], channels=128)
        gat_ps = ps.tile([P, 1024], F32, tag="ps_d")
        for c in range(7):
            kc = hk[c]
            PT = wk.tile([128, P], F32, tag="pt")
            V.tensor_scalar(PT[:kc, :], patchB[:kc, :], float(128 * c),
                            if128[:kc, :], ALU.subtract, ALU.is_equal)
            PE.matmul(gat_ps[:, 0:384], lhsT=PT[:kc, :], rhs=hid[c][:, 0:384],
                      start=(c == 0), stop=(c == 6))
            PE.matmul(gat_ps[:, 512:896], lhsT=PT[:kc, :], rhs=hid[c][:, 384:768],
                      start=(c == 0), stop=(c == 6))
        selsb = wk.tile([P, 768], F32, tag="selsb")
        V.tensor_copy(selsb[:, 0:384], gat_ps[:, 0:384])
        V.tensor_copy(selsb[:, 384:768], gat_ps[:, 512:896])
        SY.dma_start(out=d_os.ap(), in_=selsb[:])

        # ---------------- pw image, binary mask, anchor ----------------
        pw_sb = wk.tile([1, S], F32, tag="pw_sb")
        V.tensor_copy(pw_sb[:], pw_ps[:])
        pw28 = wk.tile([H28, H28], F32, tag="pw28")
        SY.dma_start(out=pw28[0:14, :], in_=pw_sb[:, 0:392])
        SY.dma_start(out=pw28[14:28, :], in_=pw_sb[:, 392:784])

        pwsum = wk.tile([H28, 1], F32, tag="pwsum")
        V.tensor_reduce(pwsum[:], pw28[:], AX.X, ALU.add)
        S1c = wk.tile([H28, 1], F32, tag="S1c")
        GP.partition_all_reduce(S1c[:], pwsum[:], channels=H28,
                                reduce_op=bass_isa.ReduceOp.add)
        binary = wk.tile([H28, H28], F32, tag="binary")
        V.tensor_scalar(binary[:], pw28[:], 784.0, S1c[:], ALU.mult, ALU.is_gt)
        mm28 = wk.tile([H28, H28], F32, tag="mm28")
        V.tensor_mul(mm28[:], pw28[:], binary[:])

        rowmax = wk.tile([H28, 1], F32, tag="rowmax")
        V.tensor_reduce(rowmax[:], mm28[:], AX.X, ALU.max)
        gmax = wk.tile([H28, 1], F32, tag="gmax")
        GP.partition_all_reduce(gmax[:], rowmax[:], channels=H28,
                                reduce_op=bass_isa.ReduceOp.max)
        eq28 = wk.tile([H28, H28], F32, tag="eq28")
        V.tensor_scalar(eq28[:], mm28[:], gmax[:], None, ALU.is_equal)

        P3 = wk.tile([H28, 3], F32, tag="P3")
        jk0 = wk.tile([H28, H28], F32, tag="jk0")
        V.scalar_tensor_tensor(jk0[:], eq28[:], 1.0, iconst[:], ALU.mult,
                               ALU.mult, accum_out=P3[:, 0:1])
        jk1 = wk.tile([H28, H28], F32, tag="jk1")
        V.scalar_tensor_tensor(jk1[:], eq28[:], 1.0, jconst[:], ALU.mult,
                               ALU.mult, accum_out=P3[:, 1:2])
        jk2 = wk.tile([H28, H28], F32, tag="jk2")
        V.scalar_tensor_tensor(jk2[:], eq28[:], 1.0, pw28[:], ALU.mult,
                               ALU.mult, accum_out=P3[:, 2:3])
        A3 = wk.tile([H28, 3], F32, tag="A3")
        GP.partition_all_reduce(A3[:], P3[:], channels=H28,
                                reduce_op=bass_isa.ReduceOp.add)

        # ---------------- structure info (dist, ang) ----------------
        di = wk.tile([H28, H28], F32, tag="di")
        V.tensor_scalar(di[:], iconst[:], A3[:, 0:1], None, ALU.subtract)
        dj = wk.tile([H28, H28], F32, tag="dj")
        V.tensor_scalar(dj[:], jconst[:], A3[:, 1:2], None, ALU.subtract)

        adi = wk.tile([H28, H28], F32, tag="adi")
        V.scalar_tensor_tensor(adi[:], di[:], -1.0, di[:], ALU.mult, ALU.max)
        adj = wk.tile([H28, H28], F32, tag="adj")
        V.scalar_tensor_tensor(adj[:], dj[:], -1.0, dj[:], ALU.mult, ALU.max)
        mn = wk.tile([H28, H28], F32, tag="mn")
        V.tensor_tensor(mn[:], adi[:], adj[:], ALU.min)
        mx = wk.tile([H28, H28], F32, tag="mx")
        V.tensor_tensor(mx[:], adi[:], adj[:], ALU.max)
        z0 = wk.tile([H28, H28], F32, tag="z0")
        V.tensor_scalar(z0[:], mx[:], 0.0, None, ALU.is_equal)
        mxs = wk.tile([H28, H28], F32, tag="mxs")
        V.tensor_add(mxs[:], mx[:], z0[:])
        rec = wk.tile([H28, H28], F32, tag="rec")
        V.reciprocal(rec[:], mxs[:])
        q28 = wk.tile([H28, H28], F32, tag="q28")
        V.tensor_mul(q28[:], mn[:], rec[:])
        base = wk.tile([H28, H28], F32, tag="base")
        SC.activation(base[:], q28[:], ACT.Arctan)
        swap = wk.tile([H28, H28], F32, tag="swap")
        V.tensor_tensor(swap[:], adj[:], adi[:], ALU.is_gt)
        u_t = wk.tile([H28, H28], F32, tag="u_t")
        V.tensor_scalar(u_t[:], base[:], -2.0, PI / 2.0, ALU.mult, ALU.add)
        us = wk.tile([H28, H28], F32, tag="us")
        V.tensor_mul(us[:], u_t[:], swap[:])
        theta = wk.tile([H28, H28], F32, tag="theta")
        V.tensor_add(theta[:], base[:], us[:])
        dineg = wk.tile([H28, H28], F32, tag="dineg")
        V.tensor_scalar(dineg[:], di[:], 0.0, None, ALU.is_lt)
        w_t = wk.tile([H28, H28], F32, tag="w_t")
        V.tensor_scalar(w_t[:], theta[:], -2.0, PI, ALU.mult, ALU.add)
        wd = wk.tile([H28, H28], F32, tag="wd")
        V.tensor_mul(wd[:], w_t[:], dineg[:])
        inner = wk.tile([H28, H28], F32, tag="inner")
        V.tensor_add(inner[:], theta[:], wd[:])
        ypos = wk.tile([H28, H28], F32, tag="ypos")
        V.tensor_scalar(ypos[:], dj[:], 0.0, None, ALU.is_ge)
        ysgn = wk.tile([H28, H28], F32, tag="ysgn")
        V.tensor_scalar(ysgn[:], ypos[:], 2.0, -1.0, ALU.mult, ALU.add)
        at2 = wk.tile([H28, H28], F32, tag="at2")
        V.tensor_mul(at2[:], inner[:], ysgn[:])
        ang28 = wk.tile([H28, H28], F32, tag="ang28")
        V.tensor_scalar(ang28[:], at2[:], 1.0 / (2.0 * PI), 0.5, ALU.mult, ALU.add)

        d2 = wk.tile([H28, H28], F32, tag="d2")
        V.tensor_mul(d2[:], di[:], di[:])
        e2 = wk.tile([H28, H28], F32, tag="e2")
        V.tensor_mul(e2[:], dj[:], dj[:])
        sum2 = wk.tile([H28, H28], F32, tag="sum2")
        V.tensor_add(sum2[:], d2[:], e2[:])
        dist28 = wk.tile([H28, H28], F32, tag="dist28")
        SC.activation(dist28[:], sum2[:], ACT.Sqrt, scale=1.0 / 784.0)

        # ---------------- GCN (collapsed) ----------------
        D3 = wk.tile([H28, 3], F32, tag="D3")
        jk3 = wk.tile([H28, H28], F32, tag="jk3")
        V.scalar_tensor_tensor(jk3[:], pw28[:], 1.0, pw28[:], ALU.mult,
                               ALU.mult, accum_out=D3[:, 0:1])
        jk4 = wk.tile([H28, H28], F32, tag="jk4")
        V.scalar_tensor_tensor(jk4[:], pw28[:], 1.0, dist28[:], ALU.mult,
                               ALU.mult, accum_out=D3[:, 1:2])
        jk5 = wk.tile([H28, H28], F32, tag="jk5")
        V.scalar_tensor_tensor(jk5[:], pw28[:], 1.0, ang28[:], ALU.mult,
                               ALU.mult, accum_out=D3[:, 2:3])
        Q3 = wk.tile([H28, 3], F32, tag="Q3")
        GP.partition_all_reduce(Q3[:], D3[:], channels=H28,
                                reduce_op=bass_isa.ReduceOp.add)

        qd2 = wk.tile([2, 1], F32, tag="qd2")
        SY.dma_start(out=qd2[:], in_=Q3[0:1, 1:3])
        u1_ps = ps.tile([1, 512], F32, tag="ps_c")
        PE.matmul(u1_ps[:], lhsT=qd2[:], rhs=g1sb[:], start=True, stop=True)
        v512 = wk.tile([1, 512], F32, tag="v512")
        SC.activation(v512[:], u1_ps[:], ACT.Relu, scale=Q3[0:1, 0:1])
        vT = wk.tile([128, 4], F32, tag="vT")
        SY.dma_start(out=vT[:], in_=v512[:].rearrange("a (c p) -> (a p) c", p=128))

        u2_ps = ps.tile([1, 1024], F32, tag="ps_a")
        for c in range(4):
            PE.matmul(u2_ps[:, 0:384], lhsT=vT[:, c:c + 1], rhs=g2c[c][:, 0:384],
                      start=(c == 0), stop=(c == 3))
            PE.matmul(u2_ps[:, 512:896], lhsT=vT[:, c:c + 1], rhs=g2c[c][:, 384:768],
                      start=(c == 0), stop=(c == 3))

        pre = wk.tile([1, 768], F32, tag="pre")
        pre_v = pre[:].rearrange("a (b f) -> a b f", b=2)
        u2_v = u2_ps[:].rearrange("a (b f) -> a b f", b=2)[:, :, 0:384]
        V.tensor_scalar(pre_v, u2_v, A3[0:1, 2:3], None, ALU.mult)
        lk = wk.tile([1, 768], F32, tag="lk")
        V.tensor_scalar(lk[:], pre[:], 0.2, None, ALU.mult)
        delta = wk.tile([1, 768], F32, tag="delta")
        V.tensor_tensor(delta[:], pre[:], lk[:], ALU.max)
        row0 = wk.tile([1, 768], F32, tag="row0")
        V.tensor_add(row0[:], hid[0][0:1, :], delta[:])
        SY.dma_start(out=d_oh.ap()[0:1, :], in_=row0[:])

    nc.compile()
    return nc


_NC_CACHE = None


def _get_nc():
    global _NC_CACHE
    if _NC_CACHE is None:
        _NC_CACHE = build_nc()
    return _NC_CACHE


def _ensure_ntff_shim():
    """bass_utils imports antenv.axon_hooks when trace=True; some images
    lack that module. Provide it (and register the boot's ctypes hook) so
    profiling works instead of crashing."""
    import sys
    import types
    try:
        import antenv.axon_hooks  # noqa: F401
        return
    except ImportError:
        pass
    mod = types.ModuleType("antenv.axon_hooks")
    _h = [None]
    mod.set_axon_ntff_profile_hook = lambda h: _h.__setitem__(0, h)
    mod.get_axon_ntff_profile_hook = lambda: _h[0]
    sys.modules["antenv.axon_hooks"] = mod
    try:
        import antenv
        antenv.axon_hooks = mod
    except ImportError:
        pass
    try:
        from trn_agent_boot.trn_boot import _ntff_profile_via_ctypes
        mod.set_axon_ntff_profile_hook(
            _ntff_profile_via_ctypes("/opt/axon/libaxon_pjrt.so"))
    except Exception:
        pass


def kernel(hidden_states, x, contribution, gc1_w, gc2_w):
    nc = _get_nc()
    hidden_states = np.ascontiguousarray(hidden_states, dtype=np.float32)
    score = np.ascontiguousarray(x[:, :, 0, 1:], dtype=np.float32)
    gc1_w = np.ascontiguousarray(gc1_w, dtype=np.float32)
    gc2_w = np.ascontiguousarray(gc2_w, dtype=np.float32)

    in_maps = []
    for b in range(B):
        in_maps.append({
            "score": score[b],
            "score96": score[b].reshape(C * NCHUNK, CH),
            "hidden": hidden_states[b],
            "gc1w": gc1_w,
            "gc2w": gc2_w,
        })
    trace = bool(os.environ.get("KERNEL_TRACE")) or bool(os.environ.get("BASS_TRACE"))
    if trace:
        _ensure_ntff_shim()
    res = run_bass_kernel_spmd(nc, in_maps, core_ids=list(range(B)), trace=trace)
    if trace and res.exec_time_ns is not None:
        print(f"HW exec time: {res.exec_time_ns} ns")
    outs = res.results
    out_h = np.stack([outs[b]["out_hidden"] for b in range(B)])
    out_s = np.stack([outs[b]["out_sel"] for b in range(B)])
    out_p = np.stack([outs[b]["out_patch"][0].astype(np.int32) for b in range(B)])
    return out_h, out_s, out_p
